# revision 1
# baseline (speedup 1.0000x reference)
"""Trainium2 Bass kernel for nn_Attentional_Aggregation (segment softmax attention).

Math (reference):
    keys_i = emb_i @ Wk.T + bk
    q_g    = emb[last(g)] @ Wq.T + bq
    logit_i = <q_{g(i)}, keys_i>
    w = segment_softmax(logit)
    out_g = sum_{i in g} w_i * keys_i

Reformulation:
    logit_i = <qk_{g(i)}, emb_i>,  qk_g = embL_g @ (Wq.T Wk) + bq Wk
    out_g = (sum e_i emb_i / sum e_i) @ Wk.T + bk   (device: Wk @ S and denom;
                                                     host: divide + bias)

Device strategy (per core, fully static SPMD program):
  phase A: qkT[c, g] = ARm.T @ embLT (+ u bias via ACT), SBUF-resident, 512-chunks.
  phase B, software-pipelined (stage1(b) || stage2(b-1)) over 98 blocks
  (128 groups, C_b element tiles of 128 each):
    stage1: plain DMAs of natural emb [i, t*129+c] bf16 (col 128 = ones) and
      host-pretransposed embT [c, t*128+i] f16; per tile logits MM
      (lhsT=embT_t, rhs=qkT_blk) into [128, CH*128] PSUM chunks; one EXP per
      chunk -> E bf16 (ACT).
    stage2: one-hot ohb = is_eq(iotab, srel bcast) + one batched multiply
      me = ohb*E (DVE); flip-scatter: spT[g, 0:129] += me_t.T @ [emb_t | 1]
      accumulates BOTH the numerator S_T[g,c] and the denominator (col 128)
      in a single MM chain per tile; den col + S_T strip copied out (DVE/ACT).
  Every 32 blocks: ONE batched SBUF->SBUF xbar transpose turns the S_T strips
  into S [c, b, g] (few xbar-mode toggles - per-block transposes serialize the
  DMA pipeline), then projections otp = Wk @ S batched 4 blocks per weight
  load; bf16 outT. Host: divide by den, add bk, un-permute groups.

Sharding: 12500 groups per core; groups bin-packed (snake by size) into 98
blocks of <=128 groups to equalize per-block element counts; per-block tile
counts C_b shared across cores (max profile). Host un-permutes the output.
"""

import os
import numpy as np
import ml_dtypes

import concourse.bacc as bacc
import concourse.bass as bass
import concourse.mybir as mybir
import concourse.tile as tile
from concourse.bass_utils import run_bass_kernel_spmd

BF16 = ml_dtypes.bfloat16
FP16 = np.float16

N = 1_000_000
G = 100_000
D = 128
NCORES = 8
NBLK = 98
GPC = G // NCORES          # groups per core (12500)
GC = NBLK * 128            # group slots per core (12544)

# Exposed for test harness
LAST_EXEC_NS = None
LAST_RESULTS = None

_cache = {}


def _build_program(C, ncores=NCORES, enable_asserts=False):
    """Build the SPMD Bass program. C = per-block tile counts (len NBLK)."""
    C = list(C)
    nblk = len(C)
    Cmax = max(C)
    assert Cmax <= 12, f"PSUM budget assumes Cmax<=12, got {Cmax}"
    tot = sum(C)
    f32 = mybir.dt.float32
    bf16 = mybir.dt.bfloat16
    f16 = mybir.dt.float16
    ts = bass.ts
    gc = nblk * 128

    nc = bacc.Bacc(
        "TRN2",
        target_bir_lowering=False,
        debug=False,
        enable_asserts=enable_asserts,
        num_devices=ncores,
    )

    # Inputs (per-core data)
    embp = nc.dram_tensor("embp", [128, tot * 129], bf16, kind="ExternalInput").ap()
    embTp = nc.dram_tensor("embTp", [128, tot * 128], f16, kind="ExternalInput").ap()
    segrel = nc.dram_tensor("segrel", [128, tot], f32, kind="ExternalInput").ap()
    embLT = nc.dram_tensor("embLT", [128, gc], f16, kind="ExternalInput").ap()
    # Constants (identical across cores)
    arm = nc.dram_tensor("arm", [128, 128], f16, kind="ExternalInput").ap()
    ucol = nc.dram_tensor("ucol", [128, 1], f32, kind="ExternalInput").ap()
    wkt = nc.dram_tensor("wkt", [128, 128], bf16, kind="ExternalInput").ap()
    iota = nc.dram_tensor("iota", [128, 128], bf16, kind="ExternalInput").ap()
    iotab = nc.dram_tensor("iotab", [128, Cmax * 128], bf16, kind="ExternalInput").ap()
    # Outputs
    outT = nc.dram_tensor("outT", [128, gc], bf16, kind="ExternalOutput").ap()
    dens = nc.dram_tensor("dens", [128, nblk], f32, kind="ExternalOutput").ap()

    # logits-psum chunking (tiles per chunk); phase A chunking
    CH = 6
    ACH = 512
    nach = (gc + ACH - 1) // ACH

    with tile.TileContext(nc) as tc:
        with (
            tc.tile_pool(name="cpool", bufs=1) as cpool,
            tc.tile_pool(name="qpsum", bufs=2, space="PSUM") as qpsum,   # shared: phase A + logits chunks (2 banks each)
            tc.tile_pool(name="bemb", bufs=5) as bemb,
            tc.tile_pool(name="bembt", bufs=5) as bembt,
            tc.tile_pool(name="bmeta", bufs=5) as bmeta,
            tc.tile_pool(name="be", bufs=4) as be,
            tc.tile_pool(name="boh", bufs=4) as boh,
            tc.tile_pool(name="bme", bufs=4) as bme,
            tc.tile_pool(name="bps", bufs=2, space="PSUM") as bps,       # 2 banks
            tc.tile_pool(name="bpo", bufs=2, space="PSUM") as bpo,       # 2 banks
            tc.tile_pool(name="bsb", bufs=2) as bsb,
        ):
            # ---- constants ----
            arm_sb = cpool.tile([128, 128], f16)
            nc.sync.dma_start(out=arm_sb[:], in_=arm)
            ucol_sb = cpool.tile([128, 1], f32)
            nc.sync.dma_start(out=ucol_sb[:], in_=ucol)
            wkt_sb = cpool.tile([128, 128], bf16)
            nc.sync.dma_start(out=wkt_sb[:], in_=wkt)
            iota_sb = cpool.tile([128, 128], bf16)
            nc.sync.dma_start(out=iota_sb[:], in_=iota)
            iotab_sb = cpool.tile([128, Cmax, 128], bf16)
            nc.sync.dma_start(out=iotab_sb[:], in_=iotab)
            embLT_sb = cpool.tile([128, gc], f16)
            nc.sync.dma_start(out=embLT_sb[:], in_=embLT)
            den_all = cpool.tile([128, nblk], f32)
            stsT_all = cpool.tile([128, gc], bf16)   # S_T strips, transposed at the end
            sts_all = cpool.tile([128, nblk, 128], bf16)
            qkT = cpool.tile([128, gc], f16)      # SBUF-resident qk table

            # ---- phase A chunks are interleaved into the block loop: block b
            # only needs qkT chunk b//4, so chunk emission rides ahead of it ----
            def phaseA_chunk(a):
                w = min(ACH, gc - a * ACH)
                qp = qpsum.tile([128, ACH], f32, space="PSUM", tag="acc",
                                padded_shape=[128, CH * 128], name=f"qp{a}")
                nc.tensor.matmul(
                    qp[:, :w], lhsT=arm_sb[:], rhs=embLT_sb[:, a * ACH : a * ACH + w],
                    start=True, stop=True,
                )
                nc.scalar.activation(
                    qkT[:, a * ACH : a * ACH + w], qp[:, :w],
                    mybir.ActivationFunctionType.Identity, bias=ucol_sb[:],
                )

            # ---- phase B (software-pipelined: stage1(b) then stage2(b-1)) ----
            offs = [0]
            for cb in C:
                offs.append(offs[-1] + cb)
            state = {}
            proj = {}   # pending blocks for the batched projection

            def stage1(b):
                cb = C[b]
                off = offs[b]
                embt = bemb.tile([128, Cmax * 129], bf16, name=f"embt{b}", tag="embt")
                nc.sync.dma_start(
                    out=embt[:, : cb * 129], in_=embp[:, off * 129 : (off + cb) * 129]
                )
                embT = bembt.tile([128, Cmax, 128], f16, name=f"embT{b}", tag="embT")
                nc.sync.dma_start(
                    out=embT[:, :cb, :], in_=embTp[:, off * 128 : (off + cb) * 128]
                )
                srel = bmeta.tile([128, Cmax], f32, name=f"srel{b}", tag="srel")
                nc.sync.dma_start(out=srel[:, :cb], in_=segrel[:, off : off + cb])

                # logits in CH-tile chunks; exp chases each chunk
                ebig = be.tile([128, Cmax, 128], bf16, name=f"ebig{b}", tag="ebig")
                for c0 in range(0, cb, CH):
                    cw = min(CH, cb - c0)
                    psumL = qpsum.tile([128, CH * 128], f32, space="PSUM", tag="acc")
                    for t in range(cw):
                        nc.tensor.matmul(
                            psumL[:, ts(t, 128)], lhsT=embT[:, c0 + t, :],
                            rhs=qkT[:, ts(b, 128)], start=True, stop=True,
                        )
                    nc.scalar.activation(
                        ebig[:, c0 : c0 + cw, :], psumL[:, : cw * 128],
                        mybir.ActivationFunctionType.Exp,
                    )
                state[b] = (embt, srel, ebig)

            XB = 32   # blocks per batched SBUF-xbar transpose

            def stage2(b):
                cb = C[b]
                embt, srel, ebig = state.pop(b)
                # one-hot mask, then one batched multiply
                ohb = boh.tile([128, Cmax, 128], bf16, name=f"ohb{b}", tag="ohb")
                nc.vector.tensor_tensor(
                    out=ohb[:, :cb, :],
                    in0=iotab_sb[:, :cb, :],
                    in1=srel[:, :cb].unsqueeze(2).broadcast_to([128, cb, 128]),
                    op=mybir.AluOpType.is_equal,
                )
                meb = bme.tile([128, Cmax, 128], bf16, name=f"meb{b}", tag="meb")
                nc.vector.tensor_tensor(
                    out=meb[:, :cb, :], in0=ohb[:, :cb, :], in1=ebig[:, :cb, :],
                    op=mybir.AluOpType.mult,
                )

                # scatter + denominator in one accumulating matmul chain:
                # spT[g, 0:128] = sum_i me[i,g] * emb[i,c];  spT[g, 128] = den[g]
                spT = bps.tile([128, 129], f32, space="PSUM", name=f"spT{b}", tag="spT")
                for t in range(cb):
                    nc.tensor.matmul(
                        spT[:], lhsT=meb[:, t, :], rhs=embt[:, t * 129 : (t + 1) * 129],
                        start=(t == 0), stop=(t == cb - 1),
                    )
                nc.vector.tensor_copy(den_all[:, b : b + 1], spT[:, 128:129])
                nc.scalar.activation(
                    stsT_all[:, ts(b, 128)], spT[:, :128],
                    mybir.ActivationFunctionType.Copy,
                )
                # batched un-transpose every XB blocks (few xbar-mode toggles),
                # then immediately project those blocks (one Wk load per 4)
                if (b + 1) % XB == 0 or b == nblk - 1:
                    b0 = (b // XB) * XB
                    nbx = b - b0 + 1
                    nc.sync.dma_start_transpose(
                        out=sts_all[:, b0 : b0 + nbx, :],
                        in_=stsT_all[:, b0 * 128 : (b0 + nbx) * 128],
                    )
                    for p0 in range(b0, b0 + nbx, 4):
                        nb = min(4, b0 + nbx - p0)
                        otp4 = bpo.tile([128, 512], f32, space="PSUM", name=f"otp{p0}", tag="otp")
                        nc.tensor.matmul(
                            otp4[:, : nb * 128], lhsT=wkt_sb[:],
                            rhs=sts_all[:, p0 : p0 + nb, :], start=True, stop=True,
                        )
                        ots4 = bsb.tile([128, 512], bf16, name=f"ots{p0}", tag="ots4")
                        nc.scalar.activation(
                            ots4[:, : nb * 128], otp4[:, : nb * 128],
                            mybir.ActivationFunctionType.Copy,
                        )
                        nc.sync.dma_start(
                            out=outT[:, p0 * 128 : (p0 + nb) * 128], in_=ots4[:, : nb * 128]
                        )

            for b in range(nblk):
                if b % 4 == 0 and b // 4 < nach:
                    phaseA_chunk(b // 4)
                stage1(b)
                if b > 0:
                    stage2(b - 1)
            stage2(nblk - 1)

            nc.sync.dma_start(out=dens, in_=den_all[:])

    nc.compile()
    return nc


def _host_prep(embeddings, seg_ids, Wq, bq, Wk, bk):
    """Bin-pack groups, build per-core arrays + constants, and the output map.

    Returns (C profile, in_maps, perm) where perm[core, slot] = global group id
    (or -1) for slot = b*128 + j.
    """
    emb = np.ascontiguousarray(embeddings, dtype=np.float32)
    seg = np.ascontiguousarray(seg_ids, dtype=np.int64)

    counts = np.bincount(seg, minlength=G)
    cum = np.concatenate([[0], np.cumsum(counts)])   # group g elements: cum[g]:cum[g+1]
    last_idx = np.cumsum(counts) - 1

    ARm = (Wq.T @ Wk).astype(np.float32)
    uvec = (bq @ Wk).astype(np.float32)

    emb_bf = emb.astype(BF16)

    # ---- bin-pack each core's groups into NBLK blocks (<=128 groups each) ----
    # snake deal by descending size, then sort blocks by load desc
    core_blocks = []       # [core][b] -> (group_ids array, load)
    for c in range(NCORES):
        g0 = c * GPC
        gids = np.arange(g0, g0 + GPC)
        sizes = counts[gids]
        order = np.argsort(-sizes, kind="stable")
        sg = gids[order]
        blocks = [[] for _ in range(NBLK)]
        loads = np.zeros(NBLK, dtype=np.int64)
        # snake deal
        pos = 0
        fwd = True
        for k in range(len(sg)):
            idx = pos if fwd else NBLK - 1 - pos
            blocks[idx].append(sg[k])
            loads[idx] += counts[sg[k]]
            pos += 1
            if pos == NBLK:
                pos = 0
                fwd = not fwd
        bo = np.argsort(-loads, kind="stable")
        core_blocks.append([(np.array(blocks[i], dtype=np.int64), int(loads[i])) for i in bo])

    # per-block tile profile shared across cores
    C = []
    for b in range(NBLK):
        mx = max(core_blocks[c][b][1] for c in range(NCORES))
        C.append(max(1, (mx + 127) // 128))
    tot = sum(C)

    iota = np.tile(np.arange(128, dtype=np.float32), (128, 1)).astype(BF16)
    Cmax = max(C)
    consts = dict(
        arm=ARm.astype(FP16),
        ucol=uvec.reshape(128, 1).astype(np.float32),
        wkt=np.ascontiguousarray(Wk.T.astype(np.float32)).astype(BF16),
        iota=iota,
        iotab=np.ascontiguousarray(np.tile(iota, (1, Cmax))),
    )

    in_maps = []
    perm = np.full((NCORES, GC), -1, dtype=np.int64)
    offs = np.concatenate([[0], np.cumsum(C)]).astype(np.int64)
    for c in range(NCORES):
        # packed group order (block-major), with per-group block id and column
        gorder = np.concatenate([core_blocks[c][b][0] for b in range(NBLK)])
        gblk = np.concatenate(
            [np.full(len(core_blocks[c][b][0]), b, dtype=np.int64) for b in range(NBLK)]
        )
        gj = np.concatenate(
            [np.arange(len(core_blocks[c][b][0]), dtype=np.int64) for b in range(NBLK)]
        )
        lens = counts[gorder]
        ne = int(lens.sum())
        # element global indices = concatenated ranges cum[g]:cum[g+1]
        lens_cum = np.concatenate([[0], np.cumsum(lens)[:-1]]).astype(np.int64)
        within = np.arange(ne, dtype=np.int64) - np.repeat(lens_cum, lens)
        eidx = np.repeat(cum[gorder], lens) + within
        eblk = np.repeat(gblk, lens)
        ej = np.repeat(gj, lens)
        # position within block (elements are in block-major order)
        blk_sizes = np.bincount(eblk, minlength=NBLK).astype(np.int64)
        blk_start = np.concatenate([[0], np.cumsum(blk_sizes)[:-1]]).astype(np.int64)
        pos = np.arange(ne, dtype=np.int64) - np.repeat(blk_start, blk_sizes)
        t = pos // 128
        i = pos % 128
        Tg = offs[eblk] + t          # global tile index in [0, tot)

        embp3 = np.zeros((128, tot, 129), dtype=BF16)
        embp3[:, :, 128] = BF16(1.0)
        embp3[i, Tg, :128] = emb_bf[eidx]
        embT3 = np.zeros((128, tot, 128), dtype=FP16)
        embT3[:, Tg, i] = emb[eidx].T.astype(FP16)
        segrel = np.full((128, tot), -1.0, dtype=np.float32)
        segrel[i, Tg] = ej.astype(np.float32)
        embLT = np.zeros((128, GC), dtype=FP16)
        embLT[:, gblk * 128 + gj] = emb[last_idx[gorder]].T.astype(FP16)
        perm[c, gblk * 128 + gj] = gorder

        m = dict(
            embp=np.ascontiguousarray(embp3.reshape(128, tot * 129)),
            embTp=np.ascontiguousarray(embT3.reshape(128, tot * 128)),
            segrel=np.ascontiguousarray(segrel),
            embLT=np.ascontiguousarray(embLT),
        )
        m.update(consts)
        in_maps.append(m)
    return C, in_maps, perm


def kernel(embeddings, seg_ids, Wq, bq, Wk, bk):
    global LAST_EXEC_NS, LAST_RESULTS
    Wq = np.asarray(Wq, dtype=np.float32)
    bq = np.asarray(bq, dtype=np.float32)
    Wk = np.asarray(Wk, dtype=np.float32)
    bk = np.asarray(bk, dtype=np.float32)
    embeddings = np.asarray(embeddings)
    seg_ids = np.asarray(seg_ids)

    C, in_maps, perm = _host_prep(embeddings, seg_ids, Wq, bq, Wk, bk)

    key = tuple(C)
    if key not in _cache:
        _cache[key] = _build_program(C)
    nc = _cache[key]

    trace = bool(int(os.environ.get("BASS_KERNEL_TRACE", "0")))
    res = run_bass_kernel_spmd(nc, in_maps, core_ids=list(range(NCORES)), trace=trace)
    LAST_RESULTS = res
    LAST_EXEC_NS = res.exec_time_ns

    out = np.empty((G, D), dtype=np.float32)
    for c in range(NCORES):
        oT = res.results[c]["outT"].astype(np.float32)     # [128, GC]
        dn = res.results[c]["dens"].T.reshape(-1)          # [128, NBLK] -> slot b*128+j
        valid = perm[c] >= 0
        out[perm[c, valid]] = oT[:, valid].T / dn[valid, None] + bk
    return out



# revision 2
# speedup vs baseline: 1.1038x; 1.1038x over previous
"""Trainium2 Bass kernel for nn_Attentional_Aggregation (segment softmax attention).

Math (reference):
    keys_i = emb_i @ Wk.T + bk
    q_g    = emb[last(g)] @ Wq.T + bq
    logit_i = <q_{g(i)}, keys_i>
    w = segment_softmax(logit)
    out_g = sum_{i in g} w_i * keys_i

Reformulation:
    logit_i = <qk_{g(i)}, emb_i>,  qk_g = embL_g @ (Wq.T Wk) + bq Wk
    out_g = (sum e_i emb_i / sum e_i) @ Wk.T + bk   (device: S = sum me emb and
                                                     den; host: divide + bias)

v2 design (windowed, rank-matched sharding):
  Groups are sorted by size globally and dealt round-robin to the 8 cores;
  each rank is padded to the max size across cores (~0.1% padding) so ALL
  cores share an identical layout structure -> one SPMD program with
  compile-time per-tile window offsets.

  Each core: 98 blocks of <=128 group slots, elements packed 128/tile.
  A tile's elements span <=32 consecutive slots (verified on host), so all
  per-element work runs at width 32 instead of 128:
    logits MM: psumL[i, t*32+j] = embT_t.T @ qkT[:, blk*128+o_t : +32]
    one EXP per block over [128, cb*32] -> E bf16
    one-hot: ohb = is_eq(iota32, srel bcast) (srel rides in embp col 128)
    me = ohb * E  (DVE, 2x bf16)
  Scatter in [d, g] orientation (free-dim window offsets are unrestricted):
    sp2[0:128, o_t:o_t+32]   += embt_t.T @ me_t      (zero-MM first)
    sp2[0:1, 128+o_t:+32+128] += ones.T @ me_t       (denominator row)
  One ACT copy per block moves sp2 [128, 256] f32 -> sts_all bf16 (S + den).
  Projection otp = Wk @ S batched 4 blocks; den DMA'd from sts_all row 0.
  No stsT transpose needed (S is already [d, g]).

  Engine placement: embp DMA on sync, embTp DMA on gpsimd (spread DGE cost),
  EXP + sts copy on ACT, is_eq/mult/phaseA-bias/ots copy on DVE.
"""

import os
import numpy as np
import ml_dtypes

import concourse.bacc as bacc
import concourse.bass as bass
import concourse.mybir as mybir
import concourse.tile as tile
from concourse.bass_utils import run_bass_kernel_spmd

BF16 = ml_dtypes.bfloat16
FP16 = np.float16

N = 1_000_000
G = 100_000
D = 128
NCORES = 8
NBLK = 98
GPC = G // NCORES          # groups (ranks) per core (12500)
GC = NBLK * 128            # group slots per core (12544)
W = 32                     # logits/scatter window width

LAST_EXEC_NS = None
LAST_RESULTS = None

_cache = {}


def _build_program(C, O, ncores=NCORES, enable_asserts=False):
    """Build the SPMD Bass program.

    C = per-block tile counts (len NBLK); O = per-tile window offsets
    (len sum(C), values in [0, 128-W], relative to the tile's block).
    """
    C = list(C)
    O = list(O)
    nblk = len(C)
    Cmax = max(C)
    assert Cmax <= 12, f"PSUM budget assumes Cmax<=12, got {Cmax}"
    tot = sum(C)
    assert len(O) == tot
    f32 = mybir.dt.float32
    bf16 = mybir.dt.bfloat16
    f16 = mybir.dt.float16
    ts = bass.ts
    gc = nblk * 128
    P = 0.003    # pacing period ("ms" units -> 3000 ns)
    BASE = 0.006

    nc = bacc.Bacc(
        "TRN2",
        target_bir_lowering=False,
        debug=False,
        enable_asserts=enable_asserts,
        num_devices=ncores,
    )

    # Inputs (per-core data)
    embp = nc.dram_tensor("embp", [128, tot * 129], bf16, kind="ExternalInput").ap()
    embTp = nc.dram_tensor("embTp", [128, tot * 128], f16, kind="ExternalInput").ap()
    embLT = nc.dram_tensor("embLT", [128, gc], f16, kind="ExternalInput").ap()
    # Constants (identical across cores)
    arm = nc.dram_tensor("arm", [128, 128], f16, kind="ExternalInput").ap()
    ucol = nc.dram_tensor("ucol", [128, 1], f32, kind="ExternalInput").ap()
    wkt = nc.dram_tensor("wkt", [128, 128], bf16, kind="ExternalInput").ap()
    iotab = nc.dram_tensor("iotab", [128, Cmax * W], bf16, kind="ExternalInput").ap()
    onecol = nc.dram_tensor("onecol", [128, 1], bf16, kind="ExternalInput").ap()
    zrow = nc.dram_tensor("zrow", [1, 256], bf16, kind="ExternalInput").ap()
    # Outputs
    outT = nc.dram_tensor("outT", [128, gc], bf16, kind="ExternalOutput").ap()
    dseg = Cmax * W
    dens = nc.dram_tensor("dens", [1, nblk * dseg], bf16, kind="ExternalOutput").ap()

    ACH = 512                       # phase A chunk width
    nach = (gc + ACH - 1) // ACH

    with tile.TileContext(nc) as tc:
        with (
            tc.tile_pool(name="cpool", bufs=1) as cpool,
            tc.tile_pool(name="plg", bufs=3, space="PSUM") as plg,     # logits psum
            tc.tile_pool(name="pbig", bufs=2, space="PSUM") as pbig,   # phaseA + proj
            tc.tile_pool(name="psc", bufs=3, space="PSUM") as psc,     # scatter sp2
            tc.tile_pool(name="bemb", bufs=7) as bemb,
            tc.tile_pool(name="bembt", bufs=7) as bembt,
            tc.tile_pool(name="be", bufs=5) as be,
            tc.tile_pool(name="boh", bufs=5) as boh,
            tc.tile_pool(name="bme", bufs=5) as bme,
            tc.tile_pool(name="bsb", bufs=2) as bsb,
            tc.tile_pool(name="bLT", bufs=4) as bLT,
        ):
            # ---- constants ----
            arm_sb = cpool.tile([128, 128], f16)
            nc.sync.dma_start(out=arm_sb[:], in_=arm)
            ucol_sb = cpool.tile([128, 1], f32)
            nc.sync.dma_start(out=ucol_sb[:], in_=ucol)
            wkt_sb = cpool.tile([128, 128], bf16)
            nc.sync.dma_start(out=wkt_sb[:], in_=wkt)
            iotab_sb = cpool.tile([128, Cmax, W], bf16)
            nc.sync.dma_start(out=iotab_sb[:], in_=iotab)
            onecol_sb = cpool.tile([128, 1], bf16)
            nc.sync.dma_start(out=onecol_sb[:], in_=onecol)
            zrow_sb = cpool.tile([1, 256], bf16)
            nc.sync.dma_start(out=zrow_sb[:], in_=zrow)
            qkT = cpool.tile([128, gc], f16)          # SBUF-resident qk table
            sts_all = cpool.tile([128, nblk, 128 + dseg], bf16)  # S[d,g] + den segs

            # ---- phase A chunks interleaved into the block loop ----
            def phaseA_chunk(a):
                w = min(ACH, gc - a * ACH)
                eLT = bLT.tile([128, ACH], f16, name=f"eLT{a}", tag="eLT")
                nc.sync.dma_start(out=eLT[:, :w], in_=embLT[:, a * ACH : a * ACH + w])
                qp = pbig.tile([128, ACH], f32, space="PSUM", tag="big",
                               name=f"qp{a}")
                nc.tensor.matmul(
                    qp[:, :w], lhsT=arm_sb[:], rhs=eLT[:, :w],
                    start=True, stop=True,
                )
                nc.vector.tensor_scalar_add(
                    out=qkT[:, a * ACH : a * ACH + w], in0=qp[:, :w],
                    scalar1=ucol_sb[:],
                )

            offs = [0]
            for cb in C:
                offs.append(offs[-1] + cb)
            state0 = {}
            state = {}

            def stage0(b):
                # DMAs issued two iterations ahead of the compute stages so
                # the tile scheduler's simulation sees inputs land early and
                # interleaves blocks instead of serializing the chain.
                cb = C[b]
                off = offs[b]
                embt = bemb.tile([128, Cmax, 129], bf16, name=f"embt{b}", tag="embt")
                nc.sync.dma_start(
                    out=embt[:, :cb, :], in_=embp[:, off * 129 : (off + cb) * 129]
                )
                embT = bembt.tile([128, Cmax, 128], f16, name=f"embT{b}", tag="embT")
                nc.sync.dma_start(
                    out=embT[:, :cb, :], in_=embTp[:, off * 128 : (off + cb) * 128]
                )
                state0[b] = (embt, embT)

            def stage1(b):
                cb = C[b]
                off = offs[b]
                embt, embT = state0.pop(b)

                # logits in one per-block psum tile; one EXP chases it
                ebig = be.tile([128, Cmax, W], bf16, name=f"ebig{b}", tag="ebig")
                psumL = plg.tile([128, Cmax * W], f32, space="PSUM", tag="lg")
                for t in range(cb):
                    o = O[off + t]
                    nc.tensor.matmul(
                        psumL[:, ts(t, W)], lhsT=embT[:, t, :],
                        rhs=qkT[:, b * 128 + o : b * 128 + o + W],
                        start=True, stop=True,
                    )
                nc.scalar.activation(
                    ebig[:, :cb, :], psumL[:, : cb * W],
                    mybir.ActivationFunctionType.Exp,
                )
                # zero the scatter psum S region early (PE idles less)
                sp2 = psc.tile([128, 128 + dseg], f32, space="PSUM",
                               name=f"sp2{b}", tag="sp2")
                nc.tensor.matmul(
                    sp2[:, :128], lhsT=zrow_sb[:, :128], rhs=zrow_sb[:, :128],
                    start=True, stop=False,
                )
                state[b] = (embt, ebig, sp2)

            state2 = {}

            def stage2a(b):
                cb = C[b]
                off = offs[b]
                embt, ebig, sp2 = state.pop(b)
                # one-hot mask, then one batched multiply
                ohb = boh.tile([128, Cmax, W], bf16, name=f"ohb{b}", tag="ohb")
                nc.vector.tensor_tensor(
                    out=ohb[:, :cb, :],
                    in0=iotab_sb[:, :cb, :],
                    in1=embt[:, :cb, 128:129].broadcast_to([128, cb, W]),
                    op=mybir.AluOpType.is_equal,
                )
                meb = bme.tile([128, Cmax, W], bf16, name=f"meb{b}", tag="meb")
                nc.vector.tensor_tensor(
                    out=meb[:, :cb, :], in0=ohb[:, :cb, :], in1=ebig[:, :cb, :],
                    op=mybir.AluOpType.mult,
                )

                # scatter into sp2 (zeroed in stage1): sp2[d, o:o+W] += embt_t.T @ me_t
                for t in range(cb):
                    o = O[off + t]
                    nc.tensor.matmul(
                        sp2[:, o : o + W], lhsT=embt[:, t, :128], rhs=meb[:, t, :],
                        start=False, stop=(t == cb - 1),
                    )
                # denominator segments, one batched MM (self-zeroing):
                # sp2[0, 128+t*W+j] = sum_i me_t[i, j]
                nc.tensor.matmul(
                    sp2[0:1, 128 : 128 + cb * W], lhsT=onecol_sb[:],
                    rhs=meb[:, :cb, :],
                    start=True, stop=True,
                )
                state2[b] = sp2

            def stage2b(b):
                cb = C[b]
                sp2 = state2.pop(b)
                # PSUM -> SBUF one iteration later than stage2a, so the
                # copy's dependency (den-MM) is already long done and it
                # never head-of-line blocks the next EXP on ACT.
                nc.scalar.activation(
                    sts_all[:, b, : 128 + cb * W], sp2[:, : 128 + cb * W],
                    mybir.ActivationFunctionType.Copy,
                )
                # batched projection every 2 blocks (spread so the PE spike
                # per boundary stays small)
                if (b + 1) % 2 == 0 or b == nblk - 1:
                    p0 = (b // 2) * 2
                    nb = b - p0 + 1
                    otp4 = pbig.tile([128, 256], f32, space="PSUM",
                                     name=f"otp{p0}", tag="big")
                    nc.tensor.matmul(
                        otp4[:, : nb * 128], lhsT=wkt_sb[:],
                        rhs=sts_all[:, p0 : p0 + nb, :128], start=True, stop=True,
                    )
                    ots4 = bsb.tile([128, 256], bf16, name=f"ots{p0}", tag="ots4")
                    nc.scalar.activation(
                        ots4[:, : nb * 128], otp4[:, : nb * 128],
                        mybir.ActivationFunctionType.Copy,
                    )
                    nc.sync.dma_start(
                        out=outT[:, p0 * 128 : (p0 + nb) * 128],
                        in_=ots4[:, : nb * 128],
                    )

            # Manual scheduling pace (compile-time only; shapes the scheduler's
            # stream order, not the runtime): stage1(k) at BASE+k*P, scatter
            # 2.5 periods later, copy 3.7, projection/phaseA in mid-period
            # slots. P must exceed the scheduler-sim's natural period so the
            # pacing (not sim readiness) dictates stream order.
            # prefetch the first blocks' data, then build the whole qk table
            # up front (PE/DVE are idle during the DMA fill anyway)
            for b in range(4):
                stage0(b)
            for a in range(nach):
                phaseA_chunk(a)
            for b in range(2, nblk + 5):
                if 4 <= b < nblk:
                    stage0(b)
                if 2 <= b < nblk + 2:
                    with tc.tile_wait_until(BASE + (b - 2) * P):
                        stage1(b - 2)
                if 4 <= b < nblk + 4:
                    with tc.tile_wait_until(BASE + (b - 2) * P + 0.5 * P):
                        stage2a(b - 4)
                if 5 <= b < nblk + 5:
                    with tc.tile_wait_until(BASE + (b - 2) * P + 0.7 * P):
                        stage2b(b - 5)

            nc.sync.dma_start(out=dens, in_=sts_all[0:1, :, 128 : 128 + dseg])

    nc.compile()
    return nc


def _host_prep(embeddings, seg_ids, Wq, bq, Wk, bk):
    """Rank-matched sharding + packing; returns (C, O, in_maps, gids, den_scale).

    gids[c, slot] = global group id for core c at slot = b*128+j (or -1).
    """
    emb = np.ascontiguousarray(embeddings, dtype=np.float32)
    seg = np.ascontiguousarray(seg_ids, dtype=np.int64)

    counts = np.bincount(seg, minlength=G)
    cum = np.concatenate([[0], np.cumsum(counts)])   # group g: cum[g]:cum[g+1]

    ARm = (Wq.T @ Wk).astype(np.float32)
    uvec = (bq @ Wk).astype(np.float32)

    # ---- rank-matched deal: global sort by size desc, round-robin to cores ----
    gorder = np.argsort(-counts, kind="stable")          # [G]
    gmat = gorder.reshape(GPC, NCORES)                   # rank k -> cores' gids
    sz = counts[gmat[:, 0]].astype(np.int64)             # padded size per rank (max)

    # ---- snake-deal ranks into NBLK blocks, then interleave big/small ----
    blocks = [[] for _ in range(NBLK)]
    pos, fwd = 0, True
    for k in range(GPC):
        idx = pos if fwd else NBLK - 1 - pos
        blocks[idx].append(k)
        pos += 1
        if pos == NBLK:
            pos = 0
            fwd = not fwd
    slot_rank = np.full(NBLK * 128, -1, dtype=np.int64)  # slot -> rank
    for b in range(NBLK):
        bl = blocks[b]                                   # ranks, desc size order
        half = (len(bl) + 1) // 2
        big, small = bl[:half], bl[half:][::-1]
        inter = []
        for i in range(half):
            inter.append(big[i])
            if i < len(small):
                inter.append(small[i])
        slot_rank[b * 128 : b * 128 + len(inter)] = inter

    loads = np.zeros(NBLK, dtype=np.int64)
    for b in range(NBLK):
        sl = slot_rank[b * 128 : (b + 1) * 128]
        loads[b] = sz[sl[sl >= 0]].sum()
    C = [max(1, int((l + 127) // 128)) for l in loads]
    tot = sum(C)
    offs = np.concatenate([[0], np.cumsum(C)]).astype(np.int64)

    # ---- shared per-position structure: slot / rank / within-run offset ----
    npos = tot * 128
    slot_of_pos = np.full(npos, -1, dtype=np.int64)
    rank_of_pos = np.full(npos, -1, dtype=np.int64)
    wrun_of_pos = np.zeros(npos, dtype=np.int64)
    for b in range(NBLK):
        sl = slot_rank[b * 128 : (b + 1) * 128]
        valid = sl >= 0
        js = np.nonzero(valid)[0]
        rk = sl[js]
        szs = sz[rk]
        base = offs[b] * 128
        run_start = base + np.concatenate([[0], np.cumsum(szs)[:-1]])
        ne = int(szs.sum())
        within = np.arange(ne, dtype=np.int64) - np.repeat(run_start - base, szs)
        p = base + np.arange(ne, dtype=np.int64)
        slot_of_pos[p] = np.repeat(b * 128 + js, szs)
        rank_of_pos[p] = np.repeat(rk, szs)
        wrun_of_pos[p] = within

    # ---- per-tile window offsets; assert span <= W ----
    O = np.zeros(tot, dtype=np.int64)
    sp = slot_of_pos.reshape(tot, 128) % 128
    has = (slot_of_pos.reshape(tot, 128) >= 0)
    for T in range(tot):
        if has[T].any():
            smin = sp[T][has[T]].min()
            smax = sp[T][has[T]].max()
            assert smax - smin + 1 <= W, f"tile {T} span {smax-smin+1} > {W}"
            O[T] = min(smin, 128 - W)

    srel_shared = np.where(
        slot_of_pos >= 0,
        (slot_of_pos % 128) - np.repeat(O, 128),
        -1,
    ).astype(np.float32)

    iota = np.tile(np.arange(W, dtype=np.float32), (128, 1)).astype(BF16)
    Cmax = max(C)
    consts = dict(
        arm=ARm.astype(FP16),
        ucol=uvec.reshape(128, 1).astype(np.float32),
        wkt=np.ascontiguousarray(Wk.T.astype(np.float32)).astype(BF16),
        iotab=np.ascontiguousarray(np.tile(iota, (1, Cmax))),
        onecol=np.ones((128, 1), dtype=BF16),
        zrow=np.zeros((1, 256), dtype=BF16),
    )

    # ---- per-core arrays ----
    t_of_pos = np.arange(npos, dtype=np.int64) // 128
    i_of_pos = np.arange(npos, dtype=np.int64) % 128
    gids = np.full((NCORES, GC), -1, dtype=np.int64)
    in_maps = []
    for c in range(NCORES):
        gid_rank = gmat[:, c]                            # rank -> gid
        gids[c][slot_rank >= 0] = gid_rank[slot_rank[slot_rank >= 0]]
        rsz = counts[gid_rank]                           # real size per rank
        real = (rank_of_pos >= 0) & (wrun_of_pos < rsz[rank_of_pos])
        eidx = cum[gid_rank[rank_of_pos[real]]] + wrun_of_pos[real]
        Tg = t_of_pos[real]
        ii = i_of_pos[real]

        embp3 = np.zeros((128, tot, 129), dtype=BF16)
        embp3[:, :, 128] = BF16(-1.0)
        embp3[ii, Tg, :128] = emb[eidx].astype(BF16)
        embp3[ii, Tg, 128] = srel_shared[real].astype(BF16)
        embT3 = np.zeros((128, tot, 128), dtype=FP16)
        embT3[:, Tg, ii] = emb[eidx].T.astype(FP16)
        embLT = np.zeros((128, GC), dtype=FP16)
        v = gids[c] >= 0
        last_e = cum[gids[c][v] + 1] - 1                 # last element of group
        embLT[:, np.nonzero(v)[0]] = emb[last_e].T.astype(FP16)

        m = dict(
            embp=np.ascontiguousarray(embp3.reshape(128, tot * 129)),
            embTp=np.ascontiguousarray(embT3.reshape(128, tot * 128)),
            embLT=np.ascontiguousarray(embLT),
        )
        m.update(consts)
        in_maps.append(m)
    return C, list(O), in_maps, gids


def kernel(embeddings, seg_ids, Wq, bq, Wk, bk):
    global LAST_EXEC_NS, LAST_RESULTS
    Wq = np.asarray(Wq, dtype=np.float32)
    bq = np.asarray(bq, dtype=np.float32)
    Wk = np.asarray(Wk, dtype=np.float32)
    bk = np.asarray(bk, dtype=np.float32)
    embeddings = np.asarray(embeddings)
    seg_ids = np.asarray(seg_ids)

    C, O, in_maps, gids = _host_prep(embeddings, seg_ids, Wq, bq, Wk, bk)

    key = (tuple(C), tuple(O))
    if key not in _cache:
        _cache[key] = _build_program(C, O)
    nc = _cache[key]

    trace = bool(int(os.environ.get("BASS_KERNEL_TRACE", "0")))
    res = run_bass_kernel_spmd(nc, in_maps, core_ids=list(range(NCORES)), trace=trace)
    LAST_RESULTS = res
    LAST_EXEC_NS = res.exec_time_ns

    # den reconstruction: dens[0, b*dseg + t*W + j] -> slot b*128 + O[t] + j
    Cmax = max(C)
    dseg = Cmax * W
    offs = np.concatenate([[0], np.cumsum(C)]).astype(np.int64)
    segidx = []   # flat position in dens -> slot index
    for b in range(NBLK):
        for t in range(C[b]):
            o = O[offs[b] + t]
            segidx.append(b * 128 + o + np.arange(W))
    segsrc = np.concatenate(
        [b * dseg + t * W + np.arange(W) for b in range(NBLK) for t in range(C[b])]
    )
    segdst = np.concatenate(segidx)

    out = np.empty((G, D), dtype=np.float32)
    for c in range(NCORES):
        oT = res.results[c]["outT"].astype(np.float32)     # [128, GC]
        ds = res.results[c]["dens"].astype(np.float32).reshape(-1)
        dn = np.zeros(GC, dtype=np.float32)
        np.add.at(dn, segdst, ds[segsrc])
        valid = gids[c] >= 0
        out[gids[c][valid]] = oT[:, valid].T / dn[valid, None] + bk
    return out
